# revision 1
# baseline (speedup 1.0000x reference)
"""nn_AttentionBlock talking-heads attention — 8-core Trainium2 Bass kernel.

Sharding: data-parallel over batch (2) x query-blocks (4) -> 8 cores.
Core c owns batch c//4, queries [(c%4)*512, (c%4+1)*512); K/V are computed
redundantly per core (no collectives needed).

Per-core on-device pipeline (all matmuls fp16 on PE, fp32 PSUM):
  1. QT = (Wq/sqrt(D))-proj of xq     -> [hd, 512]   (8 chunks [128, 512])
  2. KT = Wk-proj of xkv              -> [hd, 2048]  (8 chunks [128, 2048])
  3. V  = Wv-proj of xkv              -> [2048, hd]  (16 chunks [128, 1024])
  4. per q-block of 64 rows:
     a. logits via block-diag-paired Q stationary: out [(2h,64q), k]
     b. DMA partition-shuffle -> mixin [(16h,8q), qgrp, k]
     c. mix1 = matmul with W1-delta stationary (pre-softmax talking heads)
     d. exp on ACT (no max subtraction; fp16-safe range) + fused row-sums
     e. mix2 with (W2-delta * 1/rowsum) stationary  (post-softmax mix with
        softmax normalization folded into the stationary)
     f. PE-transpose P2 -> [k, (16i, 8q)], attend vs V, out-proj with Wo
"""

import numpy as np

import concourse.bass as bass
from concourse import bacc
import concourse.mybir as mybir
import concourse.tile as tile
from concourse.bass_utils import run_bass_kernel_spmd

B, S, C = 2, 2048, 1024
H, D = 16, 64
HD = H * D
NCORES = 8
SQ = S // 4        # 512 queries per core
QB = 64            # q-block rows
NQB = SQ // QB     # 8
NHP = H // 2       # 8 head pairs
NKC = S // 128     # 16 key chunks of 128
NKQ = S // 512     # 4 key quarters of 512
NCC = C // 128     # 8 contraction chunks
F16 = mybir.dt.float16
F32 = mybir.dt.float32
F8 = mybir.dt.float8e4

LAST_EXEC_NS = None
LAST_RESULTS = None


def _cp(nc, idx, out, in_):
    """Alternate PSUM->SBUF eviction copies between DVE and ACT."""
    if idx % 2 == 0:
        nc.vector.tensor_copy(out=out, in_=in_)
    else:
        nc.scalar.copy(out=out, in_=in_)


def _build(iters=1):
    nc = bacc.Bacc()
    xqT = nc.declare_dram_parameter("xqT", [C, SQ], F16, isOutput=False)
    xkvT = nc.declare_dram_parameter("xkvT", [C, S], F16, isOutput=False)
    wq = nc.declare_dram_parameter("wq", [C, HD], F16, isOutput=False)
    wk = nc.declare_dram_parameter("wk", [C, HD], F16, isOutput=False)
    wv = nc.declare_dram_parameter("wv", [C, HD], F16, isOutput=False)
    wo = nc.declare_dram_parameter("wo", [HD, C], F16, isOutput=False)
    w1d = nc.declare_dram_parameter("w1d", [128, 128], F16, isOutput=False)
    w2d = nc.declare_dram_parameter("w2d", [128, 128], F16, isOutput=False)
    y = nc.declare_dram_parameter("y", [SQ, C], F16, isOutput=True)

    with tile.TileContext(nc) as tc:
      for _it in range(iters):
        with (
            tc.tile_pool(name="const", bufs=1) as constp,
            tc.tile_pool(name="static", bufs=1) as statp,
            tc.tile_pool(name="lsbp", bufs=3) as lsbp,
            tc.tile_pool(name="stagep", bufs=4, space="DRAM") as stagep,
        ):
            # ---- constants ----
            w1d_sb = constp.tile([128, 128], F16, tag="w1d")
            nc.sync.dma_start(out=w1d_sb, in_=w1d[:, :])
            w2d_sb = constp.tile([128, 128], F16, tag="w2d")
            nc.sync.dma_start(out=w2d_sb, in_=w2d[:, :])

            # ---- persistent activations / weights ----
            qt_sb = [statp.tile([128, SQ], F16, tag=f"qt{i}", name=f"qt{i}") for i in range(NCC)]
            kt_sb = [statp.tile([128, S], F16, tag=f"kt{i}", name=f"kt{i}") for i in range(NCC)]
            v_sb = [statp.tile([128, HD], F16, tag=f"v{i}", name=f"v{i}") for i in range(NKC)]

            bd0 = [statp.tile([128, 128], F16, tag=f"bd{i}", name=f"bd{i}")
                   for i in range(NHP)]
            hoist_stages = []
            # ---- projections (own 2-bank psum pool; N=1024 moving) ----
            with (
                tc.tile_pool(name="proj", bufs=1) as projp,
                tc.tile_pool(name="projps", bufs=8, space="PSUM") as projps,
            ):
                xq_t = [projp.tile([128, SQ], F16, tag=f"xq{c}", name=f"xq{c}") for c in range(NCC)]
                xkv_t = [projp.tile([128, S], F16, tag=f"xkv{c}", name=f"xkv{c}") for c in range(NCC)]
                w_t = [projp.tile([128, HD], F16, tag=f"w{c}", name=f"w{c}") for c in range(NCC)]
                for c in range(NCC):
                    nc.sync.dma_start(out=xq_t[c], in_=xqT[c * 128:(c + 1) * 128, :])
                    nc.sync.dma_start(out=xkv_t[c], in_=xkvT[c * 128:(c + 1) * 128, :])
                    nc.sync.dma_start(out=w_t[c], in_=wq[c * 128:(c + 1) * 128, :])
                # QT[hd, q] = sum_c wq[c, hd-chunk].T @ xqT[c, q]
                for hc in range(NCC):
                    ps = projps.tile([128, 512], F32, tag="pp", name="pp")
                    for c in range(NCC):
                        nc.tensor.matmul(
                            ps, w_t[c][:, hc * 128:(hc + 1) * 128],
                            xq_t[c], start=(c == 0), stop=(c == NCC - 1))
                    _cp(nc, hc, qt_sb[hc], ps)
                # KT[hd, k]
                w2_t = [projp.tile([128, HD], F16, tag=f"w2{c}", name=f"w2{c}") for c in range(NCC)]
                for c in range(NCC):
                    nc.sync.dma_start(out=w2_t[c], in_=wk[c * 128:(c + 1) * 128, :])
                for hc in range(NCC):
                    for kq in range(NKQ):
                        ps = projps.tile([128, 512], F32, tag="pp", name="pp")
                        for c in range(NCC):
                            nc.tensor.matmul(
                                ps, w2_t[c][:, hc * 128:(hc + 1) * 128],
                                xkv_t[c][:, kq * 512:(kq + 1) * 512],
                                start=(c == 0), stop=(c == NCC - 1))
                        _cp(nc, kq, kt_sb[hc][:, kq * 512:(kq + 1) * 512], ps)
                # ---- hoisted logits for pair 0 (overlap bounce with V-proj) ----
                for i in range(NHP):
                    nc.vector.memset(bd0[i], 0.0)
                hoist_stages.clear()
                for hqb in range(4):
                    qsl = slice(hqb * QB, (hqb + 1) * QB)
                    for hp in range(NHP):
                        nc.vector.tensor_copy(
                            out=bd0[hp][0:64, 0:64].rearrange(
                                "p (j g) -> p j g", j=8),
                            in_=qt_sb[hp][0:64, qsl].rearrange(
                                "p (g j) -> p j g", g=8))
                        nc.vector.tensor_copy(
                            out=bd0[hp][64:128, 64:128].rearrange(
                                "p (j g) -> p j g", j=8),
                            in_=qt_sb[hp][64:128, qsl].rearrange(
                                "p (g j) -> p j g", g=8))
                    stage = stagep.tile([H, 8, 8, S], F16, tag="stage",
                                        name="stage")
                    hoist_stages.append(stage)
                    for hp in range(NHP):
                        lsb = lsbp.tile([128, S], F16, tag="lsb", name="lsb")
                        for kq in range(NKQ):
                            ps = projps.tile([128, 512], F32, tag="pp",
                                             name="pp")
                            nc.tensor.matmul(
                                ps, bd0[hp],
                                kt_sb[hp][:, kq * 512:(kq + 1) * 512],
                                start=True, stop=True)
                            _cp(nc, kq, lsb[:, kq * 512:(kq + 1) * 512], ps)
                        nc.sync.dma_start(
                            out=stage[2 * hp:2 * hp + 2].rearrange(
                                "h2 j g k -> (h2 j g) k"),
                            in_=lsb[:, :])
                    # rebuild bd0 for second hoisted block happens next loop
                # V[k, hd]
                w3_t = [projp.tile([128, HD], F16, tag=f"w3{c}", name=f"w3{c}") for c in range(NCC)]
                for c in range(NCC):
                    nc.sync.dma_start(out=w3_t[c], in_=wv[c * 128:(c + 1) * 128, :])
                for kc in range(NKC):
                    for hh in range(2):
                        ps = projps.tile([128, 512], F32, tag="pp", name="pp")
                        for c in range(NCC):
                            nc.tensor.matmul(
                                ps, xkv_t[c][:, kc * 128:(kc + 1) * 128],
                                w3_t[c][:, hh * 512:(hh + 1) * 512],
                                start=(c == 0), stop=(c == NCC - 1))
                        _cp(nc, hh, v_sb[kc][:, hh * 512:(hh + 1) * 512], ps)

            bd = bd0

            with (
                tc.tile_pool(name="ps1", bufs=4, space="PSUM") as ps1,
                tc.tile_pool(name="pst", bufs=2, space="PSUM") as pst,
                tc.tile_pool(name="psbig", bufs=1, space="PSUM") as psbig,
                tc.tile_pool(name="attn1", bufs=1) as attn1p,
                tc.tile_pool(name="small", bufs=1) as smallp,
                tc.tile_pool(name="wop", bufs=2) as wop,
            ):
                for qp in range(NQB // 2):
                    # p2t[k, half, j, i, qbp, g, qin]
                    p2t = attn1p.tile([128, 2, 8, H, 2, 8, 8], F16, tag="p2t",
                                      name="p2t")
                    for qbp in range(2):
                        qb = 2 * qp + qbp
                        qsl = slice(qb * QB, (qb + 1) * QB)
                        rs_part = smallp.tile([128, 8, NKQ], F32, tag="rs_part")
                        rs = smallp.tile([128, 8, 1], F32, tag="rs")
                        rsinv = smallp.tile([128, 8], F32, tag="rsinv")

                        for hp in range(0 if qp <= 1 else NHP):
                            # permuted cols: bd col (j*8+g) <- q-col (g*8+j):
                            # logits partitions come out in (h2, j, g) order
                            # so the DRAM stage bounce is contiguous both ways
                            nc.vector.tensor_copy(
                                out=bd[hp][0:64, 0:64].rearrange(
                                    "p (j g) -> p j g", j=8),
                                in_=qt_sb[hp][0:64, qsl].rearrange(
                                    "p (g j) -> p j g", g=8))
                            nc.vector.tensor_copy(
                                out=bd[hp][64:128, 64:128].rearrange(
                                    "p (j g) -> p j g", j=8),
                                in_=qt_sb[hp][64:128, qsl].rearrange(
                                    "p (g j) -> p j g", g=8))

                        # ---- logits; partition shuffle via a DRAM bounce ----
                        # stage[h, j, g, k] = logits[h, qb*64 + g*8 + j, k]
                        if qp <= 1:
                            stage = hoist_stages[qp * 2 + qbp]
                        else:
                            stage = stagep.tile([H, 8, 8, S], F16, tag="stage",
                                                name="stage")
                        for hp in range(0 if qp <= 1 else NHP):
                            lsb = lsbp.tile([128, S], F16, tag="lsb",
                                            name="lsb")
                            for kq in range(NKQ):
                                ps = ps1.tile([128, 512], F32, tag="ps",
                                              name="ps")
                                nc.tensor.matmul(
                                    ps, bd[hp],
                                    kt_sb[hp][:, kq * 512:(kq + 1) * 512],
                                    start=True, stop=True)
                                _cp(nc, kq,
                                    lsb[:, kq * 512:(kq + 1) * 512], ps)
                            nc.sync.dma_start(
                                out=stage[2 * hp:2 * hp + 2].rearrange(
                                    "h2 j g k -> (h2 j g) k"),
                                in_=lsb[:, :])
                        # ---- mix1+exp -> rowsum -> mix2T3, per g-pair ----
                        # P2T[k, (i,q')] = sum_{(h,q)} probs[(h,q),k] S2'[(h,q),(i,q')]
                        for gp in range(4):
                            mixin = attn1p.tile([128, 2, S], F16, tag="mixin",
                                                name="mixin", bufs=3)
                            nc.sync.dma_start(
                                out=mixin[:, :, :],
                                in_=stage[:, :, gp * 2:(gp + 1) * 2, :]
                                .rearrange("h j g k -> (h j) g k"))
                            probs = attn1p.tile([128, 2, S], F16, tag="probs",
                                                name="probs", bufs=2)
                            for g2 in range(2):
                                g = gp * 2 + g2
                                for kq in range(NKQ):
                                    ps = ps1.tile([128, 512], F32, tag="ps",
                                                  name="ps")
                                    nc.tensor.matmul(
                                        ps, w1d_sb,
                                        mixin[:, g2, kq * 512:(kq + 1) * 512],
                                        start=True, stop=True)
                                    nc.scalar.activation(
                                        out=probs[:, g2, kq * 512:(kq + 1) * 512],
                                        in_=ps,
                                        func=mybir.ActivationFunctionType.Exp,
                                        accum_out=rs_part[:, g, kq:kq + 1])
                                nc.vector.reduce_sum(
                                    out=rs[:, g, :], in_=rs_part[:, g, :],
                                    axis=mybir.AxisListType.X)
                                nc.vector.reciprocal(out=rsinv[:, g:g + 1],
                                                     in_=rs[:, g, :])
                                s2g = attn1p.tile([128, 128], F16,
                                                  tag=f"s2g{g2}",
                                                  name=f"s2g{g2}", bufs=2)
                                nc.vector.tensor_scalar_mul(
                                    s2g, w2d_sb, rsinv[:, g:g + 1])
                                for half in range(2):
                                    for jq in range(2):
                                        tp = pst.tile([128, 4, H, 8], F32,
                                                      tag="pst", name="pst")
                                        for j2 in range(4):
                                            kc = half * 8 + jq * 4 + j2
                                            nc.tensor.matmul(
                                                tp[:, j2],
                                                probs[:, g2,
                                                      kc * 128:(kc + 1) * 128],
                                                s2g, start=True, stop=True)
                                        _cp(nc, g + jq,
                                            p2t[:, half, jq * 4:(jq + 1) * 4,
                                                :, qbp, g, :],
                                            tp[:])

                    # ---- attend over the q-block pair (all 16 k-chunks) ----
                    ot_sb = smallp.tile([64, H, 2, QB], F16,
                                        tag="ot_sb", name="ot_sb")
                    for ih in range(2):
                        ot_ps = psbig.tile([64, H // 2, 2, QB], F32,
                                           tag="psbig", name="psbig")
                        for i2 in range(H // 2):
                            i = ih * (H // 2) + i2
                            for kc in range(NKC):
                                nc.tensor.matmul(
                                    ot_ps[:, i2, :, :],
                                    v_sb[kc][:, i * 64:(i + 1) * 64],
                                    p2t[:, kc // 8, kc % 8, i, :, :, :],
                                    start=(kc == 0), stop=(kc == NKC - 1))
                        _cp(nc, ih, ot_sb[:, ih * (H // 2):(ih + 1) * (H // 2)],
                            ot_ps)

                    # ---- out projection (M = 128 over the pair) ----
                    ot2 = smallp.tile([128, NCC, 2 * QB], F16, tag="ot2")
                    for h2 in range(2):
                        nc.sync.dma_start(
                            out=ot2[h2 * 64:(h2 + 1) * 64],
                            in_=ot_sb[:, h2::2, :, :])
                    wo_t = []
                    for ip in range(NCC):
                        wt = wop.tile([128, C], F16, tag="wo", name="wo")
                        nc.sync.dma_start(out=wt,
                                          in_=wo[ip * 128:(ip + 1) * 128, :])
                        wo_t.append(wt)
                    out_ps = psbig.tile([128, C], F32, tag="psbig", name="psbig")
                    for ip in range(NCC):
                        for ch in range(2):
                            nc.tensor.matmul(
                                out_ps[:, ch * 512:(ch + 1) * 512],
                                ot2[:, ip, :],
                                wo_t[ip][:, ch * 512:(ch + 1) * 512],
                                start=(ip == 0), stop=(ip == NCC - 1))
                    out_sb = smallp.tile([128, C], F16, tag="out_sb")
                    nc.vector.tensor_copy(out=out_sb, in_=out_ps)
                    nc.sync.dma_start(out=y[qp * 128:(qp + 1) * 128, :],
                                      in_=out_sb)
    nc.finalize()
    return nc


_NC_CACHE = None


def _host_prep(inputs):
    """Cast + shard the full inputs into per-core in_maps."""
    xq = np.asarray(inputs["inputs_q"], np.float32)
    xkv = np.asarray(inputs["inputs_kv"], np.float32)
    wq = (np.asarray(inputs["Wq"], np.float32).reshape(C, HD)
          / np.sqrt(np.float32(D))).astype(np.float16)
    wk = np.asarray(inputs["Wk"], np.float32).reshape(C, HD).astype(np.float16)
    wv = np.asarray(inputs["Wv"], np.float32).reshape(C, HD).astype(np.float16)
    wo = np.asarray(inputs["Wo"], np.float32).reshape(HD, C).astype(np.float16)
    w1 = np.asarray(inputs["Wth1"], np.float32)
    w2 = np.asarray(inputs["Wth2"], np.float32)
    eye8 = np.eye(8, dtype=np.float32)
    w1d = np.einsum("hi,qp->hqip", w1, eye8).reshape(128, 128).astype(np.float16)
    w2d = np.einsum("hi,qp->hqip", w2, eye8).reshape(128, 128).astype(np.float16)

    xkvT = [np.ascontiguousarray(xkv[b].T).astype(np.float16) for b in range(B)]
    xqTf = [np.ascontiguousarray(xq[b].T).astype(np.float16) for b in range(B)]
    in_maps = []
    for core in range(NCORES):
        b, qq = divmod(core, 4)
        in_maps.append({
            "xqT": np.ascontiguousarray(xqTf[b][:, qq * SQ:(qq + 1) * SQ]),
            "xkvT": xkvT[b],
            "wq": wq, "wk": wk, "wv": wv, "wo": wo,
            "w1d": w1d, "w2d": w2d,
        })
    return in_maps


def kernel(**inputs) -> np.ndarray:
    global _NC_CACHE, LAST_EXEC_NS, LAST_RESULTS
    inputs.pop("_trace", None)
    if _NC_CACHE is None:
        _NC_CACHE = _build()
    nc = _NC_CACHE
    in_maps = _host_prep(inputs)
    res = run_bass_kernel_spmd(nc, in_maps, core_ids=list(range(NCORES)))
    LAST_EXEC_NS = res.exec_time_ns
    LAST_RESULTS = res
    out = np.empty((B, S, C), np.float32)
    for core in range(NCORES):
        b, qq = divmod(core, 4)
        out[b, qq * SQ:(qq + 1) * SQ] = res.results[core]["y"].astype(np.float32)
    return out


if __name__ == "__main__":
    rng = np.random.default_rng(0)
    ins = {
        "inputs_q": rng.standard_normal((B, S, C)).astype(np.float32),
        "inputs_kv": rng.standard_normal((B, S, C)).astype(np.float32),
        "is_training": 0,
        "Wq": rng.standard_normal((C, H, D)).astype(np.float32) / 32,
        "Wk": rng.standard_normal((C, H, D)).astype(np.float32) / 32,
        "Wv": rng.standard_normal((C, H, D)).astype(np.float32) / 32,
        "Wth1": rng.standard_normal((H, H)).astype(np.float32) / 4,
        "Wth2": rng.standard_normal((H, H)).astype(np.float32) / 4,
        "Wo": rng.standard_normal((H, D, C)).astype(np.float32) / 32,
    }
    yy = kernel(**ins)
    print("kernel output", yy.shape, yy.dtype)

